# revision 1
# baseline (speedup 1.0000x reference)
"""AtomicCharge Trainium2 kernel (nn_AtomicCharge_77781857730661).

Strategy
--------
Data-parallel over atoms across 8 NeuronCores. The host packs molecules
(contiguous runs of the sorted `batch` tensor) into 1024 partition-rows
(8 cores x 128 partitions) of capacity T=2048 slots, so every molecule
lives contiguously along the free dim of one partition. x is uploaded
pre-transposed and pre-rounded to fp32r (tf32-like, full-rate PE path),
in the j-major order the device pipeline streams it.

Per core (raw bass, explicit semaphores; this walrus build allows only
one sync wait per compute instruction so waits are standalone):
  PE:  per group (2 row-pairs x 512 cols): 4x mm1 (W1^T x, fp32r) into
       [64,1024] PSUM halves, then 2x K=128 mm2 with per-pair placement
       stationaries accumulating atom_out directly into the packed
       [128,512] panel PSUM for the current block j (panels double-
       buffered across the j-outer loop).
  ACT: two [64,1024] Silu ops per group (bias b1 fused), writing the
       stacked [128,1024] hs tile via quadrant-base writes.
  DVE: per-j panel drains (+b2), then the segment phase: 4 segmented
       scans (forward + reversed via negative-stride APs) with
       host-built reset masks expand per-molecule sums, charge and
       1/count per atom:  out = ao + (charge - sum) * invcnt.

Host reassembles the 8 output grids and gathers per-atom values back.
Pipelining: x slabs triple-buffered, hs quadruple-buffered, mm2 lagged
two groups behind mm1 so PE never waits on the freshest silu.
The compile enables walrus redundant-LDWEIGHTS elimination (768 -> 382
weight loads).
HW: ~425-467 us/core on trn2 (8 cores), rel err ~2.3e-4 vs fp32 ref
(fp32r mantissa rounding).
"""
import sys

sys.path.insert(0, "/opt/trn_rl_repo")

import numpy as np

import concourse.bass as bass
from concourse import mybir
from concourse.bass_utils import run_bass_kernel_spmd
import concourse.bass_utils as _bu

# Enable walrus's redundant-LDWEIGHTS elimination (off by default in this
# stack); our mm1s reuse the same stationary 4x per group.
if not getattr(_bu, "_ldwopt_patched", False):
    _orig_run_command = _bu.run_command

    def _run_command_ldwopt(argv, **kw):
        argv = [a.replace("--enable-ldw-opt=false", "--enable-ldw-opt=true")
                for a in argv]
        return _orig_run_command(argv, **kw)

    _bu.run_command = _run_command_ldwopt
    _bu._ldwopt_patched = True

F32 = mybir.dt.float32
F32R = mybir.dt.float32r

# problem constants (hardcoded per spec)
N_ATOMS = 2_000_000
N_MOL = 50_000
D = 128      # node feature dim = SBUF partitions
H = 64       # hidden dim
NCORES = 8
R = 128      # atom-layout rows per core (partitions)
T = 2048     # slots per row
NB = 512     # free block size per matmul chunk
S = R * T    # padded atoms per core
NPAIR = R // 2
NBLK = T // NB

_NC_CACHE = {}
LAST_RUN_INFO = {}


def build_raw(D=128, H=64, R=128, T=2048, NB=512, use_silu=True):
    """j-outer pipeline: groups of 2 pairs; one [64,1024] silu per half;
    K=128 mm2; panels double-buffered across j."""
    NPAIR = R // 2
    NBLK = T // NB
    NG = NPAIR // 2             # groups per block; group = 2 pairs = 2048 atoms
    S = R * T
    WW = 2 * NB                 # 1024
    AOp = mybir.AluOpType

    nc = bass.Bass()
    # xT is laid out j-major on the host: block j, then pair k, then
    # (row 2k | row 2k+1) x 512 columns
    xT = nc.declare_dram_parameter("xT", [D, S], F32R, isOutput=False)
    W1 = nc.declare_dram_parameter("W1", [D, H], F32R, isOutput=False)
    b1s = nc.declare_dram_parameter("b1s", [D], F32, isOutput=False)
    b2 = nc.declare_dram_parameter("b2", [1], F32, isOutput=False)
    W2p = nc.declare_dram_parameter("W2p", [D, NPAIR * D], F32R, isOutput=False)
    aux = nc.declare_dram_parameter("aux", [R, 4 * T], F32, isOutput=False)
    out = nc.declare_dram_parameter("out", [R, T], F32, isOutput=True)

    from contextlib import ExitStack
    with ExitStack() as ctx:
        def sbuf(shape, dtype, name):
            return ctx.enter_context(nc.sbuf_tensor(name, shape, dtype))

        def psum(shape, name):
            return ctx.enter_context(nc.psum_tensor(name, shape, F32))

        w1 = sbuf([D, H], F32R, "w1")
        b1t = sbuf([D, 1], F32, "b1t")
        b2t = sbuf([D, 1], F32, "b2t")
        w2p = sbuf([D, NPAIR * D], F32R, "w2p")
        auxt = sbuf([R, 4 * T], F32, "auxt")
        GW = 2 * WW              # group width: 2 pairs x 1024 cols
        xp = [sbuf([D, 2 * GW], F32R, f"xp{s}") for s in range(3)]  # 2 groups each
        hs = [sbuf([D, WW], F32R, f"hs{s}") for s in range(4)]
        ao = sbuf([R, T], F32, "ao")
        FL = sbuf([R, T], F32, "FL")
        RLr = sbuf([R, T], F32, "RLr")
        CH = sbuf([R, T], F32, "CH")
        IV = sbuf([R, T], F32, "IV")

        hpA = [psum([H, WW], f"hpA{s}") for s in range(2)]   # 2 banks each
        hpB = psum([H, WW], "hpB")                           # 2 banks
        panels = [psum([R, NB], f"panel{s}") for s in range(2)]

        s_w = ctx.enter_context(nc.semaphore("s_w"))
        s_w2p = ctx.enter_context(nc.semaphore("s_w2p"))
        s_aux = ctx.enter_context(nc.semaphore("s_aux"))
        s_x = [ctx.enter_context(nc.semaphore(f"s_x{i}")) for i in range(3)]
        s_hpa = ctx.enter_context(nc.semaphore("s_hpa"))
        s_hpb = ctx.enter_context(nc.semaphore("s_hpb"))
        s_hs = ctx.enter_context(nc.semaphore("s_hs"))
        s_mm2 = ctx.enter_context(nc.semaphore("s_mm2"))
        s_pan = ctx.enter_context(nc.semaphore("s_pan"))
        s_dve = ctx.enter_context(nc.semaphore("s_dve"))
        s_epi = ctx.enter_context(nc.semaphore("s_epi"))
        s_out = ctx.enter_context(nc.semaphore("s_out"))
        block = ctx.enter_context(nc.Block())

        mAt = auxt[:, 0 * T:1 * T]
        mBrt = auxt[:, 1 * T:2 * T]
        pCHt = auxt[:, 2 * T:3 * T]
        pIVt = auxt[:, 3 * T:4 * T]

        def rev(ap):
            return bass.AP(tensor=ap.tensor, offset=ap.offset + (T - 1),
                           ap=[list(ap.ap[0]), [-1, T]])

        NGT = NBLK * NG          # total groups = 128

        # ---------------- SP: all DMA traffic ----------------
        @block.sync
        def _(sync):
            sync.dma_start(out=w1[:], in_=W1[:]).then_inc(s_w, 16)
            sync.dma_start(out=b1t[:], in_=b1s[:, None]).then_inc(s_w, 16)
            b2bc = bass.AP(tensor=b2.ap().tensor, offset=0, ap=[[0, D], [1, 1]])
            sync.dma_start(out=b2t[:], in_=b2bc).then_inc(s_w, 16)
            # xp slab v covers groups 2v, 2v+1 (2MB, j-major contiguous)
            sync.dma_start(out=xp[0][:], in_=xT[:, 0:2 * GW]).then_inc(s_x[0], 16)
            sync.dma_start(out=w2p[:], in_=W2p[:]).then_inc(s_w2p, 16)
            sync.dma_start(out=xp[1][:], in_=xT[:, 2 * GW:4 * GW]).then_inc(s_x[1], 16)
            sync.dma_start(out=xp[2][:], in_=xT[:, 4 * GW:6 * GW]).then_inc(s_x[2], 16)
            sync.dma_start(out=auxt[:], in_=aux[:]).then_inc(s_aux, 16)
            for v in range(3, NGT // 2):
                # slab slot free once both its groups' mm1a+mm1b consumed
                sync.wait_ge(s_hpb, 4 * (v - 2))
                sync.dma_start(out=xp[v % 3][:],
                               in_=xT[:, 2 * v * GW:(2 * v + 2) * GW]
                               ).then_inc(s_x[v % 3], 16)
            sync.wait_ge(s_epi, 1)
            sync.dma_start(out=out[:], in_=ao[:]).then_inc(s_out, 16)
            sync.wait_ge(s_out, 16)

        # ---------------- PE ----------------
        @block.tensor
        def _(tensor):
            tensor.wait_ge(s_w, 48)
            for g in range(NGT):
                j, gg = divmod(g, NG)
                kk = 2 * gg                 # first pair of the group
                v, half = divmod(g, 2)      # xp slab and half
                if half == 0:
                    tensor.wait_ge(s_x[v % 3], 16 * (v // 3 + 1))
                xbase = half * GW
                xslot = xp[v % 3]
                # hpA fill (dbl-buffered): WAR vs siluA(g-2)
                if g >= 2:
                    tensor.wait_ge(s_hs, 2 * (g - 2) + 1)
                nc.tensor.matmul(out=hpA[g % 2][:, 0:NB], lhsT=w1[:],
                                 rhs=xslot[:, xbase:xbase + NB],
                                 start=True, stop=True).then_inc(s_hpa, 1)
                nc.tensor.matmul(out=hpA[g % 2][:, NB:WW], lhsT=w1[:],
                                 rhs=xslot[:, xbase + 2 * NB:xbase + 3 * NB],
                                 start=True, stop=True).then_inc(s_hpa, 1)
                # hpB fill (single): WAR vs siluB(g-1); also covers mm2(g-1) rhs
                if g >= 1:
                    tensor.wait_ge(s_hs, 2 * g)
                nc.tensor.matmul(out=hpB[:, 0:NB], lhsT=w1[:],
                                 rhs=xslot[:, xbase + NB:xbase + 2 * NB],
                                 start=True, stop=True).then_inc(s_hpb, 1)
                nc.tensor.matmul(out=hpB[:, NB:WW], lhsT=w1[:],
                                 rhs=xslot[:, xbase + 3 * NB:xbase + 4 * NB],
                                 start=True, stop=True).then_inc(s_hpb, 1)
                if g >= 2:
                    gp = g - 2
                    jp, ggp = divmod(gp, NG)
                    if g == 2:
                        tensor.wait_ge(s_w2p, 16)
                    if ggp == 0 and jp >= 2:
                        tensor.wait_ge(s_pan, jp - 1)
                    for c in range(2):
                        kp = 2 * ggp + c
                        nc.tensor.matmul(
                            out=panels[jp % 2][:],
                            lhsT=w2p[:, kp * D:(kp + 1) * D],
                            rhs=hs[gp % 4][:, c * NB:(c + 1) * NB],
                            start=(ggp == 0 and c == 0),
                            stop=(ggp == NG - 1 and c == 1)).then_inc(s_mm2, 1)
            # tail: mm2s of the last two groups
            for gp in (NGT - 2, NGT - 1):
                tensor.wait_ge(s_hs, 2 * (gp + 1))
                jp, ggp = divmod(gp, NG)
                for c in range(2):
                    kp = 2 * ggp + c
                    nc.tensor.matmul(
                        out=panels[jp % 2][:],
                        lhsT=w2p[:, kp * D:(kp + 1) * D],
                        rhs=hs[gp % 4][:, c * NB:(c + 1) * NB],
                        start=(ggp == 0 and c == 0),
                        stop=(ggp == NG - 1 and c == 1)).then_inc(s_mm2, 1)

        # ---------------- ACT: two [64,1024] silus per group ------------
        @block.scalar
        def _(scalar):
            func = (mybir.ActivationFunctionType.Silu if use_silu
                    else mybir.ActivationFunctionType.Sigmoid)
            scalar.wait_ge(s_w, 48)
            for g in range(NGT):
                scalar.wait_ge(s_hpa, 2 * (g + 1))
                if g >= 4:
                    scalar.wait_ge(s_mm2, 2 * (g - 3))
                nc.scalar.activation(
                    out=hs[g % 4][0:H, :], in_=hpA[g % 2][:],
                    func=func, bias=b1t[0:H], scale=1.0,
                ).then_inc(s_hs, 1)
                scalar.wait_ge(s_hpb, 2 * (g + 1))
                nc.scalar.activation(
                    out=hs[g % 4][H:D, :], in_=hpB[:],
                    func=func, bias=b1t[0:H], scale=1.0,
                ).then_inc(s_hs, 1)

        # ---------------- DVE: panel drains per j + segment phase -------
        @block.vector
        def _(vector):
            tick = [0]

            def step(ins):
                ins.then_inc(s_dve, 1)
                tick[0] += 1
                vector.wait_ge(s_dve, tick[0])

            vector.wait_ge(s_aux, 16)
            step(nc.vector.tensor_tensor_scan(
                out=CH[:], data0=mAt, data1=pCHt,
                initial=0.0, op0=AOp.mult, op1=AOp.add))
            step(nc.vector.tensor_tensor_scan(
                out=IV[:], data0=mAt, data1=pIVt,
                initial=0.0, op0=AOp.mult, op1=AOp.add))

            vector.wait_ge(s_w, 48)
            for j in range(NBLK):
                # panel j complete after 2*NG*(j+1) mm2 incs
                vector.wait_ge(s_mm2, 2 * NG * (j + 1))
                nc.vector.tensor_scalar_add(
                    ao[:, j * NB:(j + 1) * NB], panels[j % 2][:], b2t[:]
                ).then_inc(s_pan, 1)
                tick[0] += 0
                vector.wait_ge(s_pan, j + 1)
            step(nc.vector.tensor_tensor_scan(
                out=FL[:], data0=mAt, data1=ao[:],
                initial=0.0, op0=AOp.mult, op1=AOp.add))
            step(nc.vector.tensor_tensor_scan(
                out=RLr[:], data0=mBrt, data1=rev(ao[:]),
                initial=0.0, op0=AOp.mult, op1=AOp.add))
            u = mAt
            step(nc.vector.tensor_add(u, FL[:], rev(RLr[:])))
            step(nc.vector.tensor_sub(u, u, ao[:]))
            step(nc.vector.tensor_sub(u, CH[:], u))
            step(nc.vector.tensor_mul(u, u, IV[:]))
            nc.vector.tensor_add(ao[:], ao[:], u).then_inc(s_epi, 1)

    return nc


def build_nc(use_silu=True):
    key = (use_silu,)
    if key in _NC_CACHE:
        return _NC_CACHE[key]
    nc = build_raw(D=D, H=H, R=R, T=T, NB=NB, use_silu=use_silu)
    _NC_CACHE[key] = nc
    return nc


def _pack(batch, charge):
    """Pack molecules into 1024 rows of capacity T. Returns per-atom slot
    positions and the per-core host-side input grids."""
    n = batch.shape[0]
    sizes = np.bincount(batch, minlength=N_MOL).astype(np.int64)
    nz = np.flatnonzero(sizes)           # non-empty molecules, in order
    szs = sizes[nz]
    nrows = NCORES * R

    # greedy sequential packing of molecules into rows
    row_of = np.empty(len(nz), np.int64)
    fstart = np.empty(len(nz), np.int64)
    r, f = 0, 0
    for i, sz in enumerate(szs):
        if f + sz > T:
            r += 1
            f = 0
        row_of[i] = r
        fstart[i] = f
        f += sz
    assert r < nrows, f"packing overflowed: needed {r + 1} rows > {nrows}"

    slot_start = row_of * T + fstart     # global slot of each molecule start
    # per-atom global slot: atoms of molecule i occupy slot_start[i] + 0..sz
    mol_atom_start = np.concatenate([[0], np.cumsum(szs)])[:-1]
    # batch is sorted, so atom a belongs to the idx-th non-empty molecule
    idx_of_atom = np.repeat(np.arange(len(nz)), szs)
    pos_of_atom = slot_start[idx_of_atom] + (np.arange(n) - mol_atom_start[idx_of_atom])

    # masks / placed values over all rows
    fill = np.zeros(nrows, np.int64)
    np.add.at(fill, row_of, szs)
    col = np.arange(T)
    mA = np.ones((nrows, T), np.float32)
    mA.reshape(-1)[slot_start] = 0.0
    mA[col[None, :] >= fill[:, None]] = 0.0
    slot_end = slot_start + szs - 1
    mBr = np.ones((nrows, T), np.float32)
    # reversed coords: slot (r, f) -> (r, T-1-f)
    mBr.reshape(-1)[(slot_end // T) * T + (T - 1 - (slot_end % T))] = 0.0
    # pad slots in reversed coords are cols < T - fill
    mBr[col[None, :] < (T - fill[:, None])] = 0.0

    pCH = np.zeros((nrows, T), np.float32)
    pCH.reshape(-1)[slot_start] = charge[nz]
    pIV = np.zeros((nrows, T), np.float32)
    pIV.reshape(-1)[slot_start] = (1.0 / szs).astype(np.float32)

    return pos_of_atom, mA, mBr, pCH, pIV


def _round_f32r(a):
    """Round fp32 array to fp32r (tf32-like: low 12 mantissa bits zero), RNE."""
    v = np.ascontiguousarray(a, dtype=np.float32).view(np.uint32)
    r = (v + 0x7FF + ((v >> 12) & 1)) & np.uint32(0xFFFFF000)
    return r.view(np.float32)


def kernel(x_scalar, batch, charge, W1, b1, W2, b2):
    x_scalar = np.asarray(x_scalar, dtype=np.float32)
    batch = np.asarray(batch, dtype=np.int32)
    charge = np.asarray(charge, dtype=np.float32)
    W1 = np.asarray(W1, dtype=np.float32)
    b1 = np.asarray(b1, dtype=np.float32)
    W2 = np.asarray(W2, dtype=np.float32)
    b2 = np.asarray(b2, dtype=np.float32)
    n = x_scalar.shape[0]

    # tolerate unsorted batch (reference data is sorted; this is insurance)
    order = None
    if np.any(np.diff(batch) < 0):
        order = np.argsort(batch, kind="stable")
        x_scalar = x_scalar[order]
        batch = batch[order]

    pos_of_atom, mA, mBr, pCH, pIV = _pack(batch, charge)

    # padded, packed, transposed x per core
    xpad = np.zeros((NCORES * S, D), np.float32)
    xpad[pos_of_atom] = _round_f32r(x_scalar)
    xT_cores = []
    for c in range(NCORES):
        a = xpad[c * S:(c + 1) * S].reshape(NPAIR, 2, NBLK, NB, D)
        a = a.transpose(2, 0, 1, 3, 4).reshape(S, D)   # j-major stream order
        xT_cores.append(np.ascontiguousarray(a.T))
    del xpad

    W2p = np.zeros((D, NPAIR * D), np.float32)
    for k in range(NPAIR):
        W2p[:H, k * D + 2 * k] = W2[:, 0]
        W2p[H:, k * D + 2 * k + 1] = W2[:, 0]
    W2p = _round_f32r(W2p)
    W1 = _round_f32r(W1)
    b1s = np.concatenate([b1, b1]).astype(np.float32)

    nc = build_nc(use_silu=True)
    in_maps = []
    for c in range(NCORES):
        sl = slice(c * R, (c + 1) * R)
        auxc = np.concatenate([mA[sl], mBr[sl], pCH[sl], pIV[sl]], axis=1)
        in_maps.append({
            "xT": xT_cores[c], "W1": W1, "b1s": b1s, "b2": b2, "W2p": W2p,
            "aux": np.ascontiguousarray(auxc),
        })

    import os
    trace = bool(int(os.environ.get("ATOMIC_TRACE", "0")))
    res = run_bass_kernel_spmd(nc, in_maps, list(range(NCORES)), trace=trace)
    LAST_RUN_INFO["exec_time_ns"] = getattr(res, "exec_time_ns", None)
    LAST_RUN_INFO["profile_json"] = getattr(res, "profile_json", None)

    big = np.concatenate([res.results[c]["out"].reshape(-1)
                          for c in range(NCORES)])
    at = big[pos_of_atom].astype(np.float32)
    if order is not None:
        inv = np.empty_like(order)
        inv[order] = np.arange(n)
        at = at[inv]
    return at



# revision 2
# speedup vs baseline: 1.5597x; 1.5597x over previous
"""AtomicCharge Trainium2 kernel (nn_AtomicCharge_77781857730661).

Strategy
--------
Data-parallel over atoms across 8 NeuronCores. The host packs molecules
(contiguous runs of the sorted `batch` tensor) into 1024 partition-rows
(8 cores x 128 partitions) of capacity T=2048 slots, so every molecule
lives contiguously along the free dim of one partition. x is uploaded
pre-transposed in bf16 (halves HBM traffic vs fp32), in the j-major
order the device pipeline streams it.

Per core (raw bass, explicit semaphores; this walrus build allows only
one sync wait per compute instruction so waits are standalone):
  PE:  per group (4 chunks x 512 atoms = 2048 atoms): 4x mm1 (W1^T x,
       bf16) col-tiled 2x on the PE array -- chunks 0/2 via tile (0,0)
       into hp[0:64], chunks 1/3 via tile (0,64) into hp[64:128], so
       consecutive tile-0/tile-64 matmuls stream concurrently and fill
       one [128,1024] PSUM tile per group. Then 2x K=128 mm2 with
       per-pair placement stationaries (bf16) accumulate atom_out into
       the packed [128,512] panel PSUM for block j (panels double-
       buffered across j; mm2 lagged 2 groups behind mm1).
  ACT: ONE [128,1024] Silu per group (bias b1 fused, bf16 out) -- full
       128-lane utilization, half the columns of the 2x[64,1024] split.
  DVE: per-j panel drains (+b2), then the segment phase: 4 segmented
       scans (forward + reversed via negative-stride APs) with
       host-built reset masks expand per-molecule sums, charge and
       1/count per atom:  out = ao + (charge - sum) * invcnt.

Host reassembles the 8 output grids and gathers per-atom values back.
Pipelining: x slabs triple-buffered (2 groups each), hp PSUM triple-
buffered, hs quadruple-buffered.
The compile enables walrus redundant-LDWEIGHTS elimination.
HW: target ~200 us/core on trn2 (8 cores); rel err ~2e-3 vs fp32 ref
(bf16 rounding).
"""
import sys

sys.path.insert(0, "/opt/trn_rl_repo")

import numpy as np
import ml_dtypes

import concourse.bass as bass
from concourse import mybir
from concourse.bass_utils import run_bass_kernel_spmd
import concourse.bass_utils as _bu

# Enable walrus's redundant-LDWEIGHTS elimination (off by default in this
# stack); our mm1s reuse the same stationaries within a group.
if not getattr(_bu, "_ldwopt_patched", False):
    _orig_run_command = _bu.run_command

    def _run_command_ldwopt(argv, **kw):
        argv = [a.replace("--enable-ldw-opt=false", "--enable-ldw-opt=true")
                for a in argv]
        return _orig_run_command(argv, **kw)

    _bu.run_command = _run_command_ldwopt
    _bu._ldwopt_patched = True

F32 = mybir.dt.float32
BF16 = mybir.dt.bfloat16
NP_BF16 = ml_dtypes.bfloat16

# problem constants (hardcoded per spec)
N_ATOMS = 2_000_000
N_MOL = 50_000
D = 128      # node feature dim = SBUF partitions
H = 64       # hidden dim
NCORES = 8
R = 128      # atom-layout rows per core (partitions)
T = 2048     # slots per row
NB = 512     # free block size per matmul chunk
S = R * T    # padded atoms per core
NPAIR = R // 2
NBLK = T // NB

_NC_CACHE = {}
LAST_RUN_INFO = {}


def build_raw(D=128, H=64, R=128, T=2048, NB=512, use_silu=True):
    """j-outer pipeline: groups of 2 pairs; col-tiled mm1 into one
    [128,1024] PSUM tile; one [128,1024] silu; K=128 mm2; panels
    double-buffered across j."""
    NPAIR = R // 2
    NBLK = T // NB
    NG = NPAIR // 2             # groups per block; group = 2 pairs = 2048 atoms
    S = R * T
    WW = 2 * NB                 # 1024 = hs width per group
    GW = 2 * WW                 # 2048 = xT columns per group
    AOp = mybir.AluOpType

    nc = bass.Bass()
    # xT is laid out j-major on the host: block j, then pair k, then
    # (row 2k | row 2k+1) x 512 columns
    xT = nc.declare_dram_parameter("xT", [D, S], BF16, isOutput=False)
    W1 = nc.declare_dram_parameter("W1", [D, H], BF16, isOutput=False)
    b1s = nc.declare_dram_parameter("b1s", [D], F32, isOutput=False)
    b2 = nc.declare_dram_parameter("b2", [1], F32, isOutput=False)
    W2p = nc.declare_dram_parameter("W2p", [D, NPAIR * D], BF16, isOutput=False)
    aux = nc.declare_dram_parameter("aux", [R, 4 * T], F32, isOutput=False)
    out = nc.declare_dram_parameter("out", [R, T], F32, isOutput=True)

    from contextlib import ExitStack
    with ExitStack() as ctx:
        def sbuf(shape, dtype, name):
            return ctx.enter_context(nc.sbuf_tensor(name, shape, dtype))

        def psum(shape, name):
            return ctx.enter_context(nc.psum_tensor(name, shape, F32))

        w1a = sbuf([D, H], BF16, "w1a")
        w1b = sbuf([D, H], BF16, "w1b")
        b1t = sbuf([D, 1], F32, "b1t")
        b2t = sbuf([D, 1], F32, "b2t")
        w2p = sbuf([D, NPAIR * D], BF16, "w2p")
        auxt = sbuf([R, 4 * T], F32, "auxt")
        xp = [sbuf([D, 2 * GW], BF16, f"xp{s}") for s in range(3)]  # 2 groups each
        hs = [sbuf([D, WW], BF16, f"hs{s}") for s in range(4)]
        ao = sbuf([R, T], F32, "ao")
        FL = sbuf([R, T], F32, "FL")
        RLr = sbuf([R, T], F32, "RLr")
        CH = sbuf([R, T], F32, "CH")
        IV = sbuf([R, T], F32, "IV")

        hp = [psum([D, WW], f"hp{s}") for s in range(3)]     # 2 banks each
        panels = [psum([R, NB], f"panel{s}") for s in range(2)]

        s_w = ctx.enter_context(nc.semaphore("s_w"))
        s_w2p = ctx.enter_context(nc.semaphore("s_w2p"))
        s_aux = ctx.enter_context(nc.semaphore("s_aux"))
        s_x = [ctx.enter_context(nc.semaphore(f"s_x{i}")) for i in range(3)]
        s_mm1 = ctx.enter_context(nc.semaphore("s_mm1"))
        s_hs = ctx.enter_context(nc.semaphore("s_hs"))
        s_mm2 = ctx.enter_context(nc.semaphore("s_mm2"))
        s_pan = ctx.enter_context(nc.semaphore("s_pan"))
        s_dve = ctx.enter_context(nc.semaphore("s_dve"))
        s_epi = ctx.enter_context(nc.semaphore("s_epi"))
        s_out = ctx.enter_context(nc.semaphore("s_out"))
        block = ctx.enter_context(nc.Block())

        mAt = auxt[:, 0 * T:1 * T]
        mBrt = auxt[:, 1 * T:2 * T]
        pCHt = auxt[:, 2 * T:3 * T]
        pIVt = auxt[:, 3 * T:4 * T]

        def rev(ap):
            return bass.AP(tensor=ap.tensor, offset=ap.offset + (T - 1),
                           ap=[list(ap.ap[0]), [-1, T]])

        NGT = NBLK * NG          # total groups = 128

        # ---------------- SP: all DMA traffic ----------------
        @block.sync
        def _(sync):
            sync.dma_start(out=w1a[:], in_=W1[:]).then_inc(s_w, 16)
            sync.dma_start(out=w1b[:], in_=W1[:]).then_inc(s_w, 16)
            sync.dma_start(out=b1t[:], in_=b1s[:, None]).then_inc(s_w, 16)
            b2bc = bass.AP(tensor=b2.ap().tensor, offset=0, ap=[[0, D], [1, 1]])
            sync.dma_start(out=b2t[:], in_=b2bc).then_inc(s_w, 16)
            # xp slab v covers groups 2v, 2v+1 (1MB, j-major contiguous)
            sync.dma_start(out=xp[0][:], in_=xT[:, 0:2 * GW]).then_inc(s_x[0], 16)
            sync.dma_start(out=w2p[:], in_=W2p[:]).then_inc(s_w2p, 16)
            sync.dma_start(out=xp[1][:], in_=xT[:, 2 * GW:4 * GW]).then_inc(s_x[1], 16)
            sync.dma_start(out=xp[2][:], in_=xT[:, 4 * GW:6 * GW]).then_inc(s_x[2], 16)
            sync.dma_start(out=auxt[:], in_=aux[:]).then_inc(s_aux, 16)
            for v in range(3, NGT // 2):
                # slab slot free once both its groups' mm1 quartets consumed
                sync.wait_ge(s_mm1, 2 * (v - 3) + 2)
                sync.dma_start(out=xp[v % 3][:],
                               in_=xT[:, 2 * v * GW:(2 * v + 2) * GW]
                               ).then_inc(s_x[v % 3], 16)
            sync.wait_ge(s_epi, 1)
            sync.dma_start(out=out[:], in_=ao[:]).then_inc(s_out, 16)
            sync.wait_ge(s_out, 16)

        # ---------------- PE ----------------
        @block.tensor
        def _(tensor):
            tensor.wait_ge(s_w, 64)

            def mm2_pair(gp):
                jp, ggp = divmod(gp, NG)
                for c in range(2):
                    kp = 2 * ggp + c
                    nc.tensor.matmul(
                        out=panels[jp % 2][:],
                        lhsT=w2p[:, kp * D:(kp + 1) * D],
                        rhs=hs[gp % 4][:, c * NB:(c + 1) * NB],
                        start=(ggp == 0 and c == 0),
                        stop=(ggp == NG - 1 and c == 1)).then_inc(s_mm2, 1)

            for g in range(NGT):
                v, half = divmod(g, 2)
                if half == 0:
                    tensor.wait_ge(s_x[v % 3], 16 * (v // 3 + 1))
                # WAR: hp[g%3] reused -> silu(g-3) must be done
                if g >= 3:
                    tensor.wait_ge(s_hs, g - 2)
                xbase = half * GW
                xslot = xp[v % 3]
                last = None
                for c in range(4):
                    po = 64 * (c & 1)          # chunks 0,2 -> rows 0:64; 1,3 -> 64:128
                    col = NB * (c >> 1)        # chunks 0,1 -> cols 0:512; 2,3 -> 512:
                    last = nc.tensor.matmul(
                        out=hp[g % 3][po:po + 64, col:col + NB],
                        lhsT=(w1a if po == 0 else w1b)[:],
                        rhs=xslot[:, xbase + c * NB:xbase + (c + 1) * NB],
                        start=True, stop=True,
                        tile_position=(0, po))
                last.then_inc(s_mm1, 1)
                if g >= 2:
                    gp = g - 2
                    jp, ggp = divmod(gp, NG)
                    if g == 2:
                        tensor.wait_ge(s_w2p, 16)
                    tensor.wait_ge(s_hs, gp + 1)
                    if ggp == 0 and jp >= 2:
                        tensor.wait_ge(s_pan, jp - 1)
                    mm2_pair(gp)
            # tail: mm2s of the last two groups
            for gp in (NGT - 2, NGT - 1):
                tensor.wait_ge(s_hs, gp + 1)
                mm2_pair(gp)

        # ---------------- ACT: one [128,1024] silu per group ------------
        @block.scalar
        def _(scalar):
            func = (mybir.ActivationFunctionType.Silu if use_silu
                    else mybir.ActivationFunctionType.Sigmoid)
            scalar.wait_ge(s_w, 64)
            for g in range(NGT):
                scalar.wait_ge(s_mm1, g + 1)
                # WAR: hs[g%4] reused -> mm2(g-4) must be done
                if g >= 4:
                    scalar.wait_ge(s_mm2, 2 * (g - 3))
                nc.scalar.activation(
                    out=hs[g % 4][:], in_=hp[g % 3][:],
                    func=func, bias=b1t[:], scale=1.0,
                ).then_inc(s_hs, 1)

        # ---------------- DVE: panel drains per j + segment phase -------
        @block.vector
        def _(vector):
            tick = [0]

            def step(ins):
                ins.then_inc(s_dve, 1)
                tick[0] += 1
                vector.wait_ge(s_dve, tick[0])

            vector.wait_ge(s_aux, 16)
            step(nc.vector.tensor_tensor_scan(
                out=CH[:], data0=mAt, data1=pCHt,
                initial=0.0, op0=AOp.mult, op1=AOp.add))
            step(nc.vector.tensor_tensor_scan(
                out=IV[:], data0=mAt, data1=pIVt,
                initial=0.0, op0=AOp.mult, op1=AOp.add))

            vector.wait_ge(s_w, 64)
            for j in range(NBLK):
                # panel j complete after 2*NG*(j+1) mm2 incs
                vector.wait_ge(s_mm2, 2 * NG * (j + 1))
                nc.vector.tensor_scalar_add(
                    ao[:, j * NB:(j + 1) * NB], panels[j % 2][:], b2t[:]
                ).then_inc(s_pan, 1)
                vector.wait_ge(s_pan, j + 1)
            step(nc.vector.tensor_tensor_scan(
                out=FL[:], data0=mAt, data1=ao[:],
                initial=0.0, op0=AOp.mult, op1=AOp.add))
            step(nc.vector.tensor_tensor_scan(
                out=RLr[:], data0=mBrt, data1=rev(ao[:]),
                initial=0.0, op0=AOp.mult, op1=AOp.add))
            u = mAt
            step(nc.vector.tensor_add(u, FL[:], rev(RLr[:])))
            step(nc.vector.tensor_sub(u, u, ao[:]))
            step(nc.vector.tensor_sub(u, CH[:], u))
            step(nc.vector.tensor_mul(u, u, IV[:]))
            nc.vector.tensor_add(ao[:], ao[:], u).then_inc(s_epi, 1)

    return nc


def build_nc(use_silu=True):
    key = (use_silu,)
    if key in _NC_CACHE:
        return _NC_CACHE[key]
    nc = build_raw(D=D, H=H, R=R, T=T, NB=NB, use_silu=use_silu)
    _NC_CACHE[key] = nc
    return nc


def _pack(batch, charge):
    """Pack molecules into 1024 rows of capacity T. Returns per-atom slot
    positions and the per-core host-side input grids."""
    n = batch.shape[0]
    sizes = np.bincount(batch, minlength=N_MOL).astype(np.int64)
    nz = np.flatnonzero(sizes)           # non-empty molecules, in order
    szs = sizes[nz]
    nrows = NCORES * R

    # greedy sequential packing of molecules into rows
    row_of = np.empty(len(nz), np.int64)
    fstart = np.empty(len(nz), np.int64)
    r, f = 0, 0
    for i, sz in enumerate(szs):
        if f + sz > T:
            r += 1
            f = 0
        row_of[i] = r
        fstart[i] = f
        f += sz
    assert r < nrows, f"packing overflowed: needed {r + 1} rows > {nrows}"

    slot_start = row_of * T + fstart     # global slot of each molecule start
    # per-atom global slot: atoms of molecule i occupy slot_start[i] + 0..sz
    mol_atom_start = np.concatenate([[0], np.cumsum(szs)])[:-1]
    # batch is sorted, so atom a belongs to the idx-th non-empty molecule
    idx_of_atom = np.repeat(np.arange(len(nz)), szs)
    pos_of_atom = slot_start[idx_of_atom] + (np.arange(n) - mol_atom_start[idx_of_atom])

    # masks / placed values over all rows
    fill = np.zeros(nrows, np.int64)
    np.add.at(fill, row_of, szs)
    col = np.arange(T)
    mA = np.ones((nrows, T), np.float32)
    mA.reshape(-1)[slot_start] = 0.0
    mA[col[None, :] >= fill[:, None]] = 0.0
    slot_end = slot_start + szs - 1
    mBr = np.ones((nrows, T), np.float32)
    # reversed coords: slot (r, f) -> (r, T-1-f)
    mBr.reshape(-1)[(slot_end // T) * T + (T - 1 - (slot_end % T))] = 0.0
    # pad slots in reversed coords are cols < T - fill
    mBr[col[None, :] < (T - fill[:, None])] = 0.0

    pCH = np.zeros((nrows, T), np.float32)
    pCH.reshape(-1)[slot_start] = charge[nz]
    pIV = np.zeros((nrows, T), np.float32)
    pIV.reshape(-1)[slot_start] = (1.0 / szs).astype(np.float32)

    return pos_of_atom, mA, mBr, pCH, pIV


def kernel(x_scalar, batch, charge, W1, b1, W2, b2):
    x_scalar = np.asarray(x_scalar, dtype=np.float32)
    batch = np.asarray(batch, dtype=np.int32)
    charge = np.asarray(charge, dtype=np.float32)
    W1 = np.asarray(W1, dtype=np.float32)
    b1 = np.asarray(b1, dtype=np.float32)
    W2 = np.asarray(W2, dtype=np.float32)
    b2 = np.asarray(b2, dtype=np.float32)
    n = x_scalar.shape[0]

    # tolerate unsorted batch (reference data is sorted; this is insurance)
    order = None
    if np.any(np.diff(batch) < 0):
        order = np.argsort(batch, kind="stable")
        x_scalar = x_scalar[order]
        batch = batch[order]

    pos_of_atom, mA, mBr, pCH, pIV = _pack(batch, charge)

    # padded, packed, transposed x per core (bf16)
    xpad = np.zeros((NCORES * S, D), NP_BF16)
    xpad[pos_of_atom] = x_scalar.astype(NP_BF16)
    xT_cores = []
    for c in range(NCORES):
        a = xpad[c * S:(c + 1) * S].reshape(NPAIR, 2, NBLK, NB, D)
        a = a.transpose(2, 0, 1, 3, 4).reshape(S, D)   # j-major stream order
        xT_cores.append(np.ascontiguousarray(a.T))
    del xpad

    W2p = np.zeros((D, NPAIR * D), np.float32)
    for k in range(NPAIR):
        W2p[:H, k * D + 2 * k] = W2[:, 0]
        W2p[H:, k * D + 2 * k + 1] = W2[:, 0]
    W2p = W2p.astype(NP_BF16)
    W1 = W1.astype(NP_BF16)
    b1s = np.concatenate([b1, b1]).astype(np.float32)

    nc = build_nc(use_silu=True)
    in_maps = []
    for c in range(NCORES):
        sl = slice(c * R, (c + 1) * R)
        auxc = np.concatenate([mA[sl], mBr[sl], pCH[sl], pIV[sl]], axis=1)
        in_maps.append({
            "xT": xT_cores[c], "W1": W1, "b1s": b1s, "b2": b2, "W2p": W2p,
            "aux": np.ascontiguousarray(auxc),
        })

    import os
    trace = bool(int(os.environ.get("ATOMIC_TRACE", "0")))
    res = run_bass_kernel_spmd(nc, in_maps, list(range(NCORES)), trace=trace)
    LAST_RUN_INFO["exec_time_ns"] = getattr(res, "exec_time_ns", None)
    LAST_RUN_INFO["profile_json"] = getattr(res, "profile_json", None)

    big = np.concatenate([res.results[c]["out"].reshape(-1)
                          for c in range(NCORES)])
    at = big[pos_of_atom].astype(np.float32)
    if order is not None:
        inv = np.empty_like(order)
        inv[order] = np.arange(n)
        at = at[inv]
    return at


# revision 3
# speedup vs baseline: 1.8231x; 1.1689x over previous
"""AtomicCharge Trainium2 kernel (nn_AtomicCharge_77781857730661).

Strategy
--------
Data-parallel over atoms across 8 NeuronCores. The host packs molecules
(contiguous runs of the sorted `batch` tensor) into 1024 partition-rows
(8 cores x 128 partitions) of capacity T=2048 slots, so every molecule
lives contiguously along the free dim of one partition. x is uploaded
pre-transposed in bf16 (halves HBM traffic vs fp32), in the j-major
order the device pipeline streams it. aux masks and the output travel
as bf16 too; only the f32 segment math stays on-chip.

Per core (raw bass, explicit semaphores; this walrus build allows only
one sync wait per compute instruction so waits are standalone):
  PE:  per group (4 chunks x 512 atoms = 2048 atoms): 4x mm1 (W1^T x,
       bf16) col-tiled 2x on the PE array -- chunks 0/2 via tile (0,0)
       into hp[0:64], chunks 1/3 via tile (0,64) into hp[64:128], so
       consecutive tile-0/tile-64 matmuls stream concurrently and fill
       one [128,1024] PSUM tile per group. Then 2x K=128 mm2 with
       per-pair placement stationaries (bf16) accumulate atom_out into
       the packed [128,512] panel PSUM for block j (panels double-
       buffered across j; mm2 lagged 6 groups behind mm1).
  ACT: ONE [128,1024] Silu per group (bias b1 fused, bf16 out) -- full
       128-lane utilization.
  DVE: per-j panel drains (+b2) followed by chained per-block forward
       segmented scans and Q = (CH-FL)*IV + ao*(1+IV) precompute, so
       the post-pipeline tail is only [reverse scan, mul, sub]:
       out = Q - rev(RL)*IV.

Pipelining: x slabs 4 groups each (16KB/partition DMA lines), 4-deep;
hp PSUM triple-buffered; hs 8-deep.
The compile enables walrus redundant-LDWEIGHTS elimination.
HW: target ~200 us/core on trn2 (8 cores); rel err ~4e-3 vs fp32 ref
(bf16 rounding).
"""
import sys

sys.path.insert(0, "/opt/trn_rl_repo")

import numpy as np
import ml_dtypes

import concourse.bass as bass
from concourse import mybir
from concourse.bass_utils import run_bass_kernel_spmd
import concourse.bass_utils as _bu

# Enable walrus's redundant-LDWEIGHTS elimination (off by default in this
# stack); our mm1s reuse the same stationaries within a group.
if not getattr(_bu, "_ldwopt_patched", False):
    _orig_run_command = _bu.run_command

    def _run_command_ldwopt(argv, **kw):
        argv = [a.replace("--enable-ldw-opt=false", "--enable-ldw-opt=true")
                for a in argv]
        return _orig_run_command(argv, **kw)

    _bu.run_command = _run_command_ldwopt
    _bu._ldwopt_patched = True

F32 = mybir.dt.float32
BF16 = mybir.dt.bfloat16
NP_BF16 = ml_dtypes.bfloat16

# problem constants (hardcoded per spec)
N_ATOMS = 2_000_000
N_MOL = 50_000
D = 128      # node feature dim = SBUF partitions
H = 64       # hidden dim
NCORES = 8
R = 128      # atom-layout rows per core (partitions)
T = 2048     # slots per row
NB = 512     # free block size per matmul chunk
S = R * T    # padded atoms per core
NPAIR = R // 2
NBLK = T // NB

_NC_CACHE = {}
LAST_RUN_INFO = {}


def build_raw(D=128, H=64, R=128, T=2048, NB=512, use_silu=True):
    """j-outer pipeline: groups of 2 pairs; col-tiled mm1 into one
    [128,1024] PSUM tile; one [128,1024] silu; K=128 mm2; panels
    double-buffered across j; incremental DVE epilogue."""
    NPAIR = R // 2
    NBLK = T // NB
    NG = NPAIR // 2             # groups per block; group = 2 pairs = 2048 atoms
    S = R * T
    WW = 2 * NB                 # 1024 = hs width per group
    GW = 2 * WW                 # 2048 = xT columns per group
    XPG = 4                     # groups per x slab
    NXP = 4                     # x slab buffers
    NHS = 8                     # hs buffers
    MM2_LAG = 6                 # mm2 trails mm1 by this many groups
    AOp = mybir.AluOpType

    nc = bass.Bass()
    # xT is laid out j-major on the host: block j, then pair k, then
    # (row 2k | row 2k+1) x 512 columns
    xT = nc.declare_dram_parameter("xT", [D, S], BF16, isOutput=False)
    W1 = nc.declare_dram_parameter("W1", [D, H], BF16, isOutput=False)
    b1s = nc.declare_dram_parameter("b1s", [D], F32, isOutput=False)
    b2 = nc.declare_dram_parameter("b2", [1], F32, isOutput=False)
    W2p = nc.declare_dram_parameter("W2p", [D, NPAIR * D], BF16, isOutput=False)
    aux = nc.declare_dram_parameter("aux", [R, 4 * T], BF16, isOutput=False)
    out = nc.declare_dram_parameter("out", [R, T], BF16, isOutput=True)

    from contextlib import ExitStack
    with ExitStack() as ctx:
        def sbuf(shape, dtype, name):
            return ctx.enter_context(nc.sbuf_tensor(name, shape, dtype))

        def psum(shape, name):
            return ctx.enter_context(nc.psum_tensor(name, shape, F32))

        w1a = sbuf([D, H], BF16, "w1a")
        w1b = sbuf([D, H], BF16, "w1b")
        b1t = sbuf([D, 1], F32, "b1t")
        b2t = sbuf([D, 1], F32, "b2t")
        w2p = sbuf([D, NPAIR * D], BF16, "w2p")
        auxt = sbuf([R, 4 * T], BF16, "auxt")
        xp = [sbuf([D, XPG * GW], BF16, f"xp{s}") for s in range(NXP)]
        hs = [sbuf([D, WW], BF16, f"hs{s}") for s in range(NHS)]
        ao = sbuf([R, T], F32, "ao")
        FL = sbuf([R, T], F32, "FL")
        RLr = sbuf([R, T], F32, "RLr")
        CH = sbuf([R, T], F32, "CH")
        IV = sbuf([R, T], F32, "IV")
        IVp = sbuf([R, T], F32, "IVp")
        obuf = sbuf([R, T], BF16, "obuf")

        hp = [psum([D, WW], f"hp{s}") for s in range(3)]     # 2 banks each
        panels = [psum([R, NB], f"panel{s}") for s in range(2)]

        s_w = ctx.enter_context(nc.semaphore("s_w"))
        s_w2p = ctx.enter_context(nc.semaphore("s_w2p"))
        s_aux = ctx.enter_context(nc.semaphore("s_aux"))
        s_x = [ctx.enter_context(nc.semaphore(f"s_x{i}")) for i in range(NXP)]
        s_mm1 = ctx.enter_context(nc.semaphore("s_mm1"))
        s_hs = ctx.enter_context(nc.semaphore("s_hs"))
        s_mm2 = ctx.enter_context(nc.semaphore("s_mm2"))
        s_pan = ctx.enter_context(nc.semaphore("s_pan"))
        s_dve = ctx.enter_context(nc.semaphore("s_dve"))
        s_epi = ctx.enter_context(nc.semaphore("s_epi"))
        s_out = ctx.enter_context(nc.semaphore("s_out"))
        block = ctx.enter_context(nc.Block())

        mAt = auxt[:, 0 * T:1 * T]
        mBrt = auxt[:, 1 * T:2 * T]
        pCHt = auxt[:, 2 * T:3 * T]
        pIVt = auxt[:, 3 * T:4 * T]

        def rev(ap):
            return bass.AP(tensor=ap.tensor, offset=ap.offset + (T - 1),
                           ap=[list(ap.ap[0]), [-1, T]])

        NGT = NBLK * NG          # total groups = 128
        NSLAB = NGT // XPG       # 32 x slabs
        SLW = XPG * GW           # slab width in xT columns

        # ---------------- SP: all DMA traffic ----------------
        @block.sync
        def _(sync):
            sync.dma_start(out=w1a[:], in_=W1[:]).then_inc(s_w, 16)
            sync.dma_start(out=w1b[:], in_=W1[:]).then_inc(s_w, 16)
            sync.dma_start(out=b1t[:], in_=b1s[:, None]).then_inc(s_w, 16)
            b2bc = bass.AP(tensor=b2.ap().tensor, offset=0, ap=[[0, D], [1, 1]])
            sync.dma_start(out=b2t[:], in_=b2bc).then_inc(s_w, 16)

            def xdma(v):
                sync.dma_start(out=xp[v % NXP][:],
                               in_=xT[:, v * SLW:(v + 1) * SLW]
                               ).then_inc(s_x[v % NXP], 16)

            xdma(0)
            sync.dma_start(out=w2p[:], in_=W2p[:]).then_inc(s_w2p, 16)
            xdma(1)
            xdma(2)
            xdma(3)
            for v in range(NXP, NSLAB):
                # slab slot free once its previous tenant's mm1s consumed
                sync.wait_ge(s_mm1, XPG * (v - NXP) + XPG)
                xdma(v)
                if v == NXP:
                    sync.dma_start(out=auxt[:, 0:2 * T],
                                   in_=aux[:, 0:2 * T]).then_inc(s_aux, 16)
                elif v == NXP + 1:
                    sync.dma_start(out=auxt[:, 2 * T:4 * T],
                                   in_=aux[:, 2 * T:4 * T]).then_inc(s_aux, 16)
            sync.wait_ge(s_epi, 1)
            sync.dma_start(out=out[:], in_=obuf[:]).then_inc(s_out, 16)
            sync.wait_ge(s_out, 16)

        # ---------------- PE ----------------
        @block.tensor
        def _(tensor):
            tensor.wait_ge(s_w, 64)

            def mm2_pair(gp):
                jp, ggp = divmod(gp, NG)
                for c in range(2):
                    kp = 2 * ggp + c
                    nc.tensor.matmul(
                        out=panels[jp % 2][:],
                        lhsT=w2p[:, kp * D:(kp + 1) * D],
                        rhs=hs[gp % NHS][:, c * NB:(c + 1) * NB],
                        start=(ggp == 0 and c == 0),
                        stop=(ggp == NG - 1 and c == 1)).then_inc(s_mm2, 1)

            for g in range(NGT):
                v, ph = divmod(g, XPG)
                if ph == 0:
                    tensor.wait_ge(s_x[v % NXP], 16 * (v // NXP + 1))
                # WAR: hp[g%3] reused -> silu(g-3) must be done
                if g >= 3:
                    tensor.wait_ge(s_hs, g - 2)
                xbase = ph * GW
                xslot = xp[v % NXP]
                last = None
                for c in range(4):
                    po = 64 * (c & 1)          # chunks 0,2 -> rows 0:64; 1,3 -> 64:128
                    col = NB * (c >> 1)        # chunks 0,1 -> cols 0:512; 2,3 -> 512:
                    last = nc.tensor.matmul(
                        out=hp[g % 3][po:po + 64, col:col + NB],
                        lhsT=(w1a if po == 0 else w1b)[:],
                        rhs=xslot[:, xbase + c * NB:xbase + (c + 1) * NB],
                        start=True, stop=True,
                        tile_position=(0, po))
                last.then_inc(s_mm1, 1)
                if g >= MM2_LAG:
                    gp = g - MM2_LAG
                    jp, ggp = divmod(gp, NG)
                    if g == MM2_LAG:
                        tensor.wait_ge(s_w2p, 16)
                    tensor.wait_ge(s_hs, gp + 1)
                    if ggp == 0 and jp >= 2:
                        tensor.wait_ge(s_pan, jp - 1)
                    mm2_pair(gp)
            # tail: mm2s of the last MM2_LAG groups
            for gp in range(NGT - MM2_LAG, NGT):
                tensor.wait_ge(s_hs, gp + 1)
                jp, ggp = divmod(gp, NG)
                if ggp == 0 and jp >= 2:
                    tensor.wait_ge(s_pan, jp - 1)
                mm2_pair(gp)

        # ---------------- ACT: one [128,1024] silu per group ------------
        @block.scalar
        def _(scalar):
            func = (mybir.ActivationFunctionType.Silu if use_silu
                    else mybir.ActivationFunctionType.Sigmoid)
            scalar.wait_ge(s_w, 64)
            for g in range(NGT):
                scalar.wait_ge(s_mm1, g + 1)
                # WAR: hs[g%NHS] reused -> mm2(g-NHS) must be done
                if g >= NHS:
                    scalar.wait_ge(s_mm2, 2 * (g - NHS + 1))
                nc.scalar.activation(
                    out=hs[g % NHS][:], in_=hp[g % 3][:],
                    func=func, bias=b1t[:], scale=1.0,
                ).then_inc(s_hs, 1)

        # ---------------- DVE: drains + incremental epilogue ------------
        @block.vector
        def _(vector):
            tick = [0]

            def step(ins):
                ins.then_inc(s_dve, 1)
                tick[0] += 1
                vector.wait_ge(s_dve, tick[0])

            vector.wait_ge(s_aux, 32)
            step(nc.vector.tensor_tensor_scan(
                out=CH[:], data0=mAt, data1=pCHt,
                initial=0.0, op0=AOp.mult, op1=AOp.add))
            step(nc.vector.tensor_tensor_scan(
                out=IV[:], data0=mAt, data1=pIVt,
                initial=0.0, op0=AOp.mult, op1=AOp.add))
            step(nc.vector.tensor_scalar_add(IVp[:], IV[:], 1.0))

            vector.wait_ge(s_w, 64)
            for j in range(NBLK):
                lo, hi = j * NB, (j + 1) * NB
                # panel j complete after 2*NG*(j+1) mm2 incs
                vector.wait_ge(s_mm2, 2 * NG * (j + 1))
                nc.vector.tensor_scalar_add(
                    ao[:, lo:hi], panels[j % 2][:], b2t[:]
                ).then_inc(s_pan, 1)
                vector.wait_ge(s_pan, j + 1)
                # chained forward segmented scan for this block
                init = 0.0 if j == 0 else FL[:, lo - 1:lo]
                step(nc.vector.tensor_tensor_scan(
                    out=FL[:, lo:hi], data0=mAt[:, lo:hi], data1=ao[:, lo:hi],
                    initial=init, op0=AOp.mult, op1=AOp.add))
                # Q_j = (CH - FL)*IV + ao*(1+IV), accumulated into CH
                step(nc.vector.tensor_sub(CH[:, lo:hi], CH[:, lo:hi], FL[:, lo:hi]))
                step(nc.vector.tensor_mul(CH[:, lo:hi], CH[:, lo:hi], IV[:, lo:hi]))
                step(nc.vector.tensor_mul(RLr[:, lo:hi], ao[:, lo:hi], IVp[:, lo:hi]))
                step(nc.vector.tensor_add(CH[:, lo:hi], CH[:, lo:hi], RLr[:, lo:hi]))
            # tail: reverse scan + 2 elementwise
            step(nc.vector.tensor_tensor_scan(
                out=RLr[:], data0=mBrt, data1=rev(ao[:]),
                initial=0.0, op0=AOp.mult, op1=AOp.add))
            step(nc.vector.tensor_mul(IV[:], rev(RLr[:]), IV[:]))
            nc.vector.tensor_sub(obuf[:], CH[:], IV[:]).then_inc(s_epi, 1)

    return nc


def build_nc(use_silu=True):
    key = (use_silu,)
    if key in _NC_CACHE:
        return _NC_CACHE[key]
    nc = build_raw(D=D, H=H, R=R, T=T, NB=NB, use_silu=use_silu)
    _NC_CACHE[key] = nc
    return nc


def _pack(batch, charge):
    """Pack molecules into 1024 rows of capacity T. Returns per-atom slot
    positions and the per-core host-side input grids."""
    n = batch.shape[0]
    sizes = np.bincount(batch, minlength=N_MOL).astype(np.int64)
    nz = np.flatnonzero(sizes)           # non-empty molecules, in order
    szs = sizes[nz]
    nrows = NCORES * R

    # greedy sequential packing of molecules into rows
    row_of = np.empty(len(nz), np.int64)
    fstart = np.empty(len(nz), np.int64)
    r, f = 0, 0
    for i, sz in enumerate(szs):
        if f + sz > T:
            r += 1
            f = 0
        row_of[i] = r
        fstart[i] = f
        f += sz
    assert r < nrows, f"packing overflowed: needed {r + 1} rows > {nrows}"

    slot_start = row_of * T + fstart     # global slot of each molecule start
    # per-atom global slot: atoms of molecule i occupy slot_start[i] + 0..sz
    mol_atom_start = np.concatenate([[0], np.cumsum(szs)])[:-1]
    # batch is sorted, so atom a belongs to the idx-th non-empty molecule
    idx_of_atom = np.repeat(np.arange(len(nz)), szs)
    pos_of_atom = slot_start[idx_of_atom] + (np.arange(n) - mol_atom_start[idx_of_atom])

    # masks / placed values over all rows
    fill = np.zeros(nrows, np.int64)
    np.add.at(fill, row_of, szs)
    col = np.arange(T)
    mA = np.ones((nrows, T), np.float32)
    mA.reshape(-1)[slot_start] = 0.0
    mA[col[None, :] >= fill[:, None]] = 0.0
    slot_end = slot_start + szs - 1
    mBr = np.ones((nrows, T), np.float32)
    # reversed coords: slot (r, f) -> (r, T-1-f)
    mBr.reshape(-1)[(slot_end // T) * T + (T - 1 - (slot_end % T))] = 0.0
    # pad slots in reversed coords are cols < T - fill
    mBr[col[None, :] < (T - fill[:, None])] = 0.0

    pCH = np.zeros((nrows, T), np.float32)
    pCH.reshape(-1)[slot_start] = charge[nz]
    pIV = np.zeros((nrows, T), np.float32)
    pIV.reshape(-1)[slot_start] = (1.0 / szs).astype(np.float32)

    return pos_of_atom, mA, mBr, pCH, pIV


def kernel(x_scalar, batch, charge, W1, b1, W2, b2):
    x_scalar = np.asarray(x_scalar, dtype=np.float32)
    batch = np.asarray(batch, dtype=np.int32)
    charge = np.asarray(charge, dtype=np.float32)
    W1 = np.asarray(W1, dtype=np.float32)
    b1 = np.asarray(b1, dtype=np.float32)
    W2 = np.asarray(W2, dtype=np.float32)
    b2 = np.asarray(b2, dtype=np.float32)
    n = x_scalar.shape[0]

    # tolerate unsorted batch (reference data is sorted; this is insurance)
    order = None
    if np.any(np.diff(batch) < 0):
        order = np.argsort(batch, kind="stable")
        x_scalar = x_scalar[order]
        batch = batch[order]

    pos_of_atom, mA, mBr, pCH, pIV = _pack(batch, charge)

    # padded, packed, transposed x per core (bf16)
    xpad = np.zeros((NCORES * S, D), NP_BF16)
    xpad[pos_of_atom] = x_scalar.astype(NP_BF16)
    xT_cores = []
    for c in range(NCORES):
        a = xpad[c * S:(c + 1) * S].reshape(NPAIR, 2, NBLK, NB, D)
        a = a.transpose(2, 0, 1, 3, 4).reshape(S, D)   # j-major stream order
        xT_cores.append(np.ascontiguousarray(a.T))
    del xpad

    W2p = np.zeros((D, NPAIR * D), np.float32)
    for k in range(NPAIR):
        W2p[:H, k * D + 2 * k] = W2[:, 0]
        W2p[H:, k * D + 2 * k + 1] = W2[:, 0]
    W2p = W2p.astype(NP_BF16)
    W1 = W1.astype(NP_BF16)
    b1s = np.concatenate([b1, b1]).astype(np.float32)

    nc = build_nc(use_silu=True)
    in_maps = []
    for c in range(NCORES):
        sl = slice(c * R, (c + 1) * R)
        auxc = np.concatenate([mA[sl], mBr[sl], pCH[sl], pIV[sl]],
                              axis=1).astype(NP_BF16)
        in_maps.append({
            "xT": xT_cores[c], "W1": W1, "b1s": b1s, "b2": b2, "W2p": W2p,
            "aux": np.ascontiguousarray(auxc),
        })

    import os
    trace = bool(int(os.environ.get("ATOMIC_TRACE", "0")))
    res = run_bass_kernel_spmd(nc, in_maps, list(range(NCORES)), trace=trace)
    LAST_RUN_INFO["exec_time_ns"] = getattr(res, "exec_time_ns", None)
    LAST_RUN_INFO["profile_json"] = getattr(res, "profile_json", None)

    big = np.concatenate([res.results[c]["out"].reshape(-1).astype(np.float32)
                          for c in range(NCORES)])
    at = big[pos_of_atom]
    if order is not None:
        inv = np.empty_like(order)
        inv[order] = np.arange(n)
        at = at[inv]
    return at
